# revision 24
# baseline (speedup 1.0000x reference)
"""Trainium2 Bass kernel for geodesic convolution (gnn_message_passing).

Reference computation (per mesh vertex m, M=50000, n_in=n_out=32, grid 5x8):
  1. pullback[m,k,:] = sum_t bc_weights[m,k,t] * signal[bc_indices[m,k,t],:]
  2. x_grid[m,b,:]   = sum_{k: rad*8+ang==b} pullback[m,k,:]
  3. out_pre[m,r,o]  = sum_{i,j,n} x_grid[m,(i,j),n] * kernel[i,(j+r)%8,o,n]
  4. out[m,o]        = max_r relu(out_pre[m,r,o])

Data-parallel over m on 8 cores (6272 padded rows each, 49 tiles of 128
vertices), raw-bass pipeline (manual semaphores; the Ant dma_gather ucode is
incompatible with TileContext's event-semaphore machinery). Per tile:
  - ONE `dma_gather` (SWDGE ucode) of all 15360 (vertex, slot) indices
    fetches, per slot, the 128-byte fp16 row-PAIR holding the indexed signal
    row (pair id = idx>>1 fits the ucode's int16 index limit; rows are
    stored padded to 256B stride since the ucode's stride field is in 256B
    units; the wanted half is selected by folding idx&1 into per-half
    duplicated fp16 barycentric weights). One call per tile amortizes the
    ~1us fixed SWDGE descriptor-generation cost per call.
  - DVE: one tensor_tensor mult by the (slot, half) weights broadcast over
    32 channels, then ONE tensor_reduce folding (tap, half) -> f32 x_grid.
  - PE: per 128-wide contraction chunk, 1 transpose-matmul (m,(b,n)) ->
    ((b,n),m); then a 10-chunk K-accumulated fp16 matmul against the
    precomputed rotated-kernel matrix W2 (1280, 256) in (o, r) column
    order; ACT copies PSUM->SBUF (casting fp16 for the matmul lhsT).
  - DVE: tensor_reduce(max) over the 8 rotations straight out of PSUM
    (relu commutes with max; ACT applies it on the reduced [128,32] tile).

The grid scatter (step 2) is folded into the gather ordering: slots are
binned host-side by their (rad, ang) cell; round p gathers the p-th slot of
every bin (dummies gather pair 0 with weight 0). For the reference's meshgrid
rad/ang layout this is the identity ordering and R=1 (fast path). R>1 uses a
slower multi-round accumulation path.
"""

import os
from contextlib import ExitStack

import numpy as np

import concourse.bacc as bacc
import concourse.bass as bass
import concourse.mybir as mybir
from concourse.bass_utils import run_bass_kernel_spmd

M, N_IN, N_OUT = 50000, 32, 32
N_RHO, N_THETA = 5, 8
KV = N_RHO * N_THETA            # 40 grid bins
NS = KV * 3                     # 120 gather slots per vertex
N_CORES = 8
TILE_M = 128
TILES_PER_CORE = 49             # 49*128 = 6272 >= ceil(50000/8)
M_CORE = TILES_PER_CORE * TILE_M
M_PAD = N_CORES * M_CORE        # 50176
NCHUNK = (KV * N_IN) // 128     # 10 contraction chunks of 128
ROT_OUT = N_THETA * N_OUT       # 256
NIDX = TILE_M * NS              # 15360 gather indices per tile
IDXF = NIDX // 16               # 960 idx free-dim (16-partition wrap)
IDXP = 32                       # idx partitions read by the queue-0 ucode

f32 = mybir.dt.float32
f16 = mybir.dt.float16
i16 = mybir.dt.int16

last_exec_time_ns = None
last_result = None

_program_cache = {}


def _raw_gather(gp, out_ap, in_ap, idxs_ap, num_idxs, elem_size,
                stride_bytes_256):
    """dma_gather emitter without bass's elem_size%256 gate.

    The 256B element floor is a transpose-mode (xbar spray) restriction;
    the non-transpose ucode handles any elem_size, only the source row
    STRIDE must be a 256B multiple (stride_bytes_256 ISA field).
    """
    nb = gp.bass
    inst = gp.add_instruction(
        mybir.InstDMAGatherAnt(
            name=nb.get_next_instruction_name(),
            ins=[*gp.lower_ap_dma(in_ap, for_custom_bir_dma=True),
                 gp.lower_ap(idxs_ap),
                 gp.lower_val_access(gp.to_reg(num_idxs))],
            outs=[gp.lower_ap(out_ap)],
            transpose=False,
            num_idxs=num_idxs,
            elem_size=elem_size,
            stride_bytes_256=stride_bytes_256,
            gen_mode=0,
            single_packet=False,
            queue_num=0,
            sbuf_tokens_per_rank=0,
            sbuf_free_dim_per_rank=0,
            sbuf_free_dim_pad_per_rank=0,
            sbuf_byte_offset=0,
        ))
    return inst


def _build_program(n_rounds: int, n_tiles: int):
    nc = bacc.Bacc("TRN2", target_bir_lowering=False, debug=False,
                   num_devices=N_CORES, dynamic_dma_scratch_size=32768)

    R = n_rounds
    # signal: fp16 row-pairs padded to 256B stride (128B payload + 128B pad)
    sig_d = nc.dram_tensor("signal", [M // 2, 4 * N_IN], f16,
                           kind="ExternalInput")
    idx_d = nc.dram_tensor("idx", [R, n_tiles, IDXP, IDXF], i16,
                           kind="ExternalInput")
    wts_d = nc.dram_tensor("wts", [R, n_tiles, TILE_M, 2 * NS], f16,
                           kind="ExternalInput")
    w2_d = nc.dram_tensor("w2", [128, NCHUNK, ROT_OUT], f16,
                          kind="ExternalInput")
    ident_d = nc.dram_tensor("identity", [128, 128], f16, kind="ExternalInput")
    out_d = nc.dram_tensor("out", [n_tiles, TILE_M, N_OUT], f32,
                           kind="ExternalOutput")

    Ns = n_tiles * R  # gather steps

    with ExitStack() as ctx:
        e = ctx.enter_context

        def sb(name, shape, dt=f32):
            return e(nc.sbuf_tensor(name, shape, dt))

        gbuf = [sb(f"g{i}", [TILE_M, NS, 2 * N_IN], f16) for i in range(2)]
        idxb = [sb(f"idx{i}", [IDXP, IDXF], i16) for i in range(2)]
        wtsb = [sb(f"wts{i}", [TILE_M, 2 * NS], f16) for i in range(2)]
        xgb = [sb(f"xg{i}", [TILE_M, KV, N_IN], f16) for i in range(2)]
        if R > 1:
            xtmp = sb("xtmp", [TILE_M, KV, N_IN], f16)
        xtb = [sb(f"xt{i}", [128, NCHUNK, 128], f16) for i in range(2)]
        rtb = [sb(f"rt{i}", [TILE_M, N_OUT]) for i in range(2)]
        otb = [sb(f"ot{i}", [TILE_M, N_OUT]) for i in range(2)]
        w2sb = sb("w2sb", [128, NCHUNK, ROT_OUT], f16)
        ident = sb("ident", [128, 128], f16)
        pstb = [e(nc.psum_tensor(f"pst{i}", [128, 128], f16)) for i in range(2)]
        opsb = [e(nc.psum_tensor(f"ops{i}", [TILE_M, ROT_OUT], f32))
                for i in range(2)]

        block = e(nc.Block())
        s_idx = [e(nc.semaphore(f"s_idx{i}")) for i in range(2)]
        s_wts = [e(nc.semaphore(f"s_wts{i}")) for i in range(2)]
        s_g = [e(nc.semaphore(f"s_g{i}")) for i in range(2)]
        s_out = [e(nc.semaphore(f"s_out{i}")) for i in range(2)]
        s_mult = e(nc.semaphore("s_mult"))
        s_red = e(nc.semaphore("s_red"))    # fold done (gbuf free, xg ready)
        s_tp = e(nc.semaphore("s_tp"))      # per-chunk transpose done
        s_xt = e(nc.semaphore("s_xt"))      # per-chunk PSUM->SBUF copy done
        s_mm = e(nc.semaphore("s_mm"))      # main matmul done
        s_rm = e(nc.semaphore("s_rm"))      # rotation-max done (ops free)
        s_relu = e(nc.semaphore("s_relu"))  # relu done (out tile ready)
        s_w2 = e(nc.semaphore("s_w2"))
        s_id = e(nc.semaphore("s_id"))

        def w(eng, sem, val):
            if val > 0:
                eng.wait_ge(sem, val)

        # ---- SP sequencer: input + output DMA ----
        @block.sync
        def _(sp):
            sp.dma_start(out=w2sb[:], in_=w2_d[:]).then_inc(s_w2, 16)
            sp.dma_start(out=ident[:], in_=ident_d[:]).then_inc(s_id, 16)
            for q in range(Ns):
                t, r = divmod(q, R)
                # idx buf q%2: gather q-2 must have retired
                w(sp, s_g[q % 2], 16 * (q // 2))
                sp.dma_start(out=idxb[q % 2][:], in_=idx_d[r, t]
                             ).then_inc(s_idx[q % 2], 16)
                # wts buf q%2: mult q-2 must be done
                w(sp, s_mult, q - 1)
                sp.dma_start(out=wtsb[q % 2][:], in_=wts_d[r, t]
                             ).then_inc(s_wts[q % 2], 16)
                if r == R - 1 and t >= 2:
                    t_o = t - 2
                    w(sp, s_relu, t_o + 1)
                    sp.dma_start(out=out_d[t_o], in_=otb[t_o % 2][:]
                                 ).then_inc(s_out[t_o % 2], 16)
            for t_o in (n_tiles - 2, n_tiles - 1):
                w(sp, s_relu, t_o + 1)
                sp.dma_start(out=out_d[t_o], in_=otb[t_o % 2][:]
                             ).then_inc(s_out[t_o % 2], 16)
            sp.wait_ge(s_out[0], 16 * ((n_tiles + 1) // 2))
            sp.wait_ge(s_out[1], 16 * (n_tiles // 2))

        # ---- Pool: gathers ----
        @block.gpsimd
        def _(gp):
            for q in range(Ns):
                w(gp, s_idx[q % 2], 16 * (q // 2 + 1))
                # g buf q%2 free (fold q-2 done; s_red incs once per q)
                w(gp, s_red, q - 1)
                _raw_gather(gp, gbuf[q % 2][:, :, :], sig_d[:, 0:2 * N_IN],
                            idxb[q % 2][:, :], NIDX, 2 * N_IN,
                            stride_bytes_256=1
                            ).then_inc(s_g[q % 2], 16)

        # ---- DVE: weight mult, (tap, half) fold, rotation max ----
        @block.vector
        def _(dv):
            for q in range(Ns):
                t, r = divmod(q, R)
                g = gbuf[q % 2]
                w(dv, s_g[q % 2], 16 * (q // 2 + 1))
                w(dv, s_wts[q % 2], 16 * (q // 2 + 1))
                gv = g[:].rearrange("p s (h n) -> p (s h) n", n=N_IN)
                wb = wtsb[q % 2][:].to_broadcast([TILE_M, 2 * NS, N_IN])
                nc.vector.tensor_tensor(out=gv, in0=gv, in1=wb,
                                        op=mybir.AluOpType.mult
                                        ).then_inc(s_mult, 1)
                # fold (t, h) -> x_grid chunk
                g5 = g[:].rearrange("p (b t) (h n) -> p b t h n", t=3, n=N_IN)
                if r == 0:
                    w(dv, s_tp, NCHUNK * (t - 1))  # xg buf free
                if R == 1:
                    # contiguous-slice fold: h in place, then t into f32 xg
                    # (beats one strided-inner tensor_reduce by ~1.5x)
                    nc.vector.tensor_tensor(
                        out=g5[:, :, :, 0, :], in0=g5[:, :, :, 0, :],
                        in1=g5[:, :, :, 1, :], op=mybir.AluOpType.add)
                    nc.vector.tensor_tensor(
                        out=xgb[t % 2][:], in0=g5[:, :, 0, 0, :],
                        in1=g5[:, :, 1, 0, :], op=mybir.AluOpType.add)
                    nc.vector.tensor_tensor(
                        out=xgb[t % 2][:], in0=xgb[t % 2][:],
                        in1=g5[:, :, 2, 0, :], op=mybir.AluOpType.add
                        ).then_inc(s_red, 1)
                else:
                    dst = xgb[t % 2] if r == 0 else xtmp
                    nc.vector.tensor_tensor(
                        out=g5[:, :, :, 0, :], in0=g5[:, :, :, 0, :],
                        in1=g5[:, :, :, 1, :], op=mybir.AluOpType.add)
                    nc.vector.tensor_tensor(
                        out=dst[:], in0=g5[:, :, 0, 0, :],
                        in1=g5[:, :, 1, 0, :], op=mybir.AluOpType.add)
                    ins = nc.vector.tensor_tensor(
                        out=dst[:], in0=dst[:],
                        in1=g5[:, :, 2, 0, :], op=mybir.AluOpType.add)
                    if r > 0:
                        ins = nc.vector.tensor_tensor(
                            out=xgb[t % 2][:], in0=xgb[t % 2][:],
                            in1=xtmp[:], op=mybir.AluOpType.add)
                    ins.then_inc(s_red, 1)
                if r == R - 1 and t >= 1:
                    tm = t - 1
                    dv.wait_ge(s_mm, tm + 1)
                    w(dv, s_relu, tm - 1)  # rt buf free
                    nc.vector.tensor_reduce(
                        out=rtb[tm % 2][:],
                        in_=opsb[tm % 2][:].rearrange("p (o r) -> p o r",
                                                      r=N_THETA),
                        axis=mybir.AxisListType.X,
                        op=mybir.AluOpType.max).then_inc(s_rm, 1)
            tm = n_tiles - 1
            dv.wait_ge(s_mm, tm + 1)
            w(dv, s_relu, tm - 1)
            nc.vector.tensor_reduce(
                out=rtb[tm % 2][:],
                in_=opsb[tm % 2][:].rearrange("p (o r) -> p o r", r=N_THETA),
                axis=mybir.AxisListType.X,
                op=mybir.AluOpType.max).then_inc(s_rm, 1)

        # ---- PE: transpose + matmuls ----
        @block.tensor
        def _(pe):
            pe.wait_ge(s_id, 16)
            pe.wait_ge(s_w2, 16)
            for t in range(n_tiles):
                xg2 = xgb[t % 2][:].rearrange("p k n -> p (k n)")
                # fold of tile t fully done
                w(pe, s_red, R * (t + 1))
                for c in range(NCHUNK):
                    G = NCHUNK * t + c
                    pst = pstb[G % 2]
                    w(pe, s_xt, G - 1)  # pst free: copy of chunk G-2 done
                    nc.tensor.matmul(
                        out=pst[:], lhsT=xg2[:, c * 128:(c + 1) * 128],
                        rhs=ident[:], is_transpose=True,
                        start=True, stop=True).then_inc(s_tp, 1)
                w(pe, s_xt, NCHUNK * (t + 1))
                w(pe, s_rm, t - 1)  # ops buf free
                ops = opsb[t % 2]
                for c in range(NCHUNK):
                    ins = nc.tensor.matmul(out=ops[:], lhsT=xtb[t % 2][:, c, :],
                                           rhs=w2sb[:, c, :],
                                           start=(c == 0),
                                           stop=(c == NCHUNK - 1))
                    if c == NCHUNK - 1:
                        ins.then_inc(s_mm, 1)

        # ---- ACT: PSUM->SBUF copies (f32 -> fp16 cast) + relu ----
        @block.scalar
        def _(ac):
            for t in range(n_tiles):
                for c in range(NCHUNK):
                    G = NCHUNK * t + c
                    w(ac, s_tp, G + 1)
                    w(ac, s_mm, t - 1)  # xt buf free
                    nc.scalar.copy(out=xtb[t % 2][:, c, :],
                                   in_=pstb[G % 2][:]).then_inc(s_xt, 1)
                if t >= 1:
                    tm = t - 1
                    w(ac, s_rm, tm + 1)
                    w(ac, s_out[tm % 2], 16 * (tm // 2))  # ot buf free
                    nc.scalar.activation(
                        out=otb[tm % 2][:], in_=rtb[tm % 2][:],
                        func=mybir.ActivationFunctionType.Relu
                        ).then_inc(s_relu, 1)
            tm = n_tiles - 1
            w(ac, s_rm, tm + 1)
            w(ac, s_out[tm % 2], 16 * (tm // 2))
            nc.scalar.activation(
                out=otb[tm % 2][:], in_=rtb[tm % 2][:],
                func=mybir.ActivationFunctionType.Relu).then_inc(s_relu, 1)

    nc.compile()
    return nc


def _build_w2(kernel):
    # W2[(i*8+j)*32+n, o*8+r] = kernel[i, (j+r)%8, o, n]
    # (o, r) column order so the rotation axis is innermost for the
    # DVE tensor_reduce(max) over r.
    k_rot = np.stack([np.roll(kernel, -r, axis=1) for r in range(N_THETA)], axis=0)
    w2 = k_rot.transpose(1, 2, 4, 3, 0).reshape(KV * N_IN, ROT_OUT)
    return np.ascontiguousarray(
        w2.reshape(NCHUNK, 128, ROT_OUT).transpose(1, 0, 2)).astype(np.float16)


def _build_rounds(bc_indices, bc_weights, rad_idx, ang_idx):
    flat = rad_idx.astype(np.int64) * N_THETA + ang_idx.astype(np.int64)
    if np.array_equal(flat, np.broadcast_to(np.arange(KV), flat.shape)):
        return (np.ascontiguousarray(bc_indices, dtype=np.int32)[None],
                np.ascontiguousarray(bc_weights, dtype=np.float32)[None])
    order = np.argsort(flat, axis=1, kind="stable")
    fs = np.take_along_axis(flat, order, axis=1)
    pos = np.broadcast_to(np.arange(KV), fs.shape)
    is_start = np.ones_like(fs, dtype=bool)
    is_start[:, 1:] = fs[:, 1:] != fs[:, :-1]
    start_pos = np.maximum.accumulate(np.where(is_start, pos, 0), axis=1)
    rank = (pos - start_pos).astype(np.int64)
    n_rounds = int(rank.max()) + 1
    bi_s = np.take_along_axis(bc_indices, order[:, :, None], axis=1)
    bw_s = np.take_along_axis(bc_weights, order[:, :, None], axis=1)
    m = flat.shape[0]
    gidx = np.zeros((n_rounds, m, KV, 3), dtype=np.int32)
    gw = np.zeros((n_rounds, m, KV, 3), dtype=np.float32)
    mm = np.broadcast_to(np.arange(m)[:, None], fs.shape)
    gidx[rank.ravel(), mm.ravel(), fs.ravel()] = bi_s.reshape(-1, 3)
    gw[rank.ravel(), mm.ravel(), fs.ravel()] = bw_s.reshape(-1, 3)
    return gidx, gw


def _prep_inputs(gidx, gw):
    """(R, M, KV, 3) idx/weights -> device idx16 (16-wrap, 2 replicas) +
    dual-half fp16 weights: idx16 (n_cores, R, n_tiles, IDXP, IDXF) i16,
    wts (n_cores, R, n_tiles, 128, 240) f16."""
    n_rounds = gidx.shape[0]
    gidx_p = np.zeros((n_rounds, M_PAD, NS), dtype=np.int32)
    gw_p = np.zeros((n_rounds, M_PAD, NS), dtype=np.float32)
    gidx_p[:, :M] = gidx.reshape(n_rounds, M, NS)
    gw_p[:, :M] = gw.reshape(n_rounds, M, NS)

    pair = (gidx_p >> 1).astype(np.int16)
    half = (gidx_p & 1).astype(np.float32)
    wts = np.empty((n_rounds, M_PAD, NS, 2), dtype=np.float32)
    wts[..., 0] = gw_p * (1.0 - half)
    wts[..., 1] = gw_p * half
    wts = wts.reshape(n_rounds, N_CORES, TILES_PER_CORE, TILE_M, 2 * NS)
    wts = np.ascontiguousarray(wts.transpose(1, 0, 2, 3, 4)).astype(np.float16)

    # gather order i = s*128 + m -> per-tile flat list (NS, 128)
    pair = pair.reshape(n_rounds, N_CORES, TILES_PER_CORE, TILE_M, NS)
    idx_flat = pair.transpose(1, 0, 2, 4, 3).reshape(
        N_CORES, n_rounds, TILES_PER_CORE, NIDX)
    # 16-partition wrap, replicated to the 2 groups of 16 partitions the
    # queue-0 ucode cores read
    wrap = idx_flat.reshape(N_CORES, n_rounds, TILES_PER_CORE, IDXF, 16)
    wrap = wrap.transpose(0, 1, 2, 4, 3)  # (.., 16, IDXF)
    idx16 = np.ascontiguousarray(
        np.broadcast_to(wrap[:, :, :, None, :, :],
                        (N_CORES, n_rounds, TILES_PER_CORE, IDXP // 16, 16,
                         IDXF))
        .reshape(N_CORES, n_rounds, TILES_PER_CORE, IDXP, IDXF))
    return idx16, wts


def kernel(signal, kernel, bc_weights, bc_indices, rad_idx, ang_idx):
    global last_exec_time_ns, last_result
    signal = np.asarray(signal, dtype=np.float32)
    # fp16 row-pairs padded to 256B stride: [25000, 128] f16, payload [:, :64]
    sig_pairs = np.zeros((M // 2, 4 * N_IN), dtype=np.float16)
    sig_pairs[:, :2 * N_IN] = signal.reshape(M // 2, 2 * N_IN).astype(np.float16)
    w2 = _build_w2(np.asarray(kernel, dtype=np.float32))
    gidx, gw = _build_rounds(np.asarray(bc_indices), np.asarray(bc_weights),
                             np.asarray(rad_idx), np.asarray(ang_idx))
    n_rounds = gidx.shape[0]
    idx16, wts = _prep_inputs(gidx, gw)

    key = (n_rounds, TILES_PER_CORE)
    if key not in _program_cache:
        _program_cache[key] = _build_program(n_rounds, TILES_PER_CORE)
    nc = _program_cache[key]

    ident = np.eye(128, dtype=np.float16)
    in_maps = [{"signal": sig_pairs, "idx": idx16[c], "wts": wts[c], "w2": w2,
                "identity": ident}
               for c in range(N_CORES)]

    trace = bool(int(os.environ.get("BASS_KERNEL_TRACE", "0")))
    kwargs = {}
    if trace:
        import prof_shim
        prof_shim.install()
        tdir = os.environ.get("BASS_KERNEL_TRACE_DIR")
        if tdir:
            os.makedirs(tdir, exist_ok=True)
            kwargs["tmpdir"] = tdir
    res = run_bass_kernel_spmd(nc, in_maps, core_ids=list(range(N_CORES)),
                               trace=trace, **kwargs)
    last_result = res
    last_exec_time_ns = res.exec_time_ns

    out = np.concatenate([res.results[c]["out"].reshape(M_CORE, N_OUT)
                          for c in range(N_CORES)], axis=0)
    return np.ascontiguousarray(out[:M])


# revision 29
# speedup vs baseline: 1.0038x; 1.0038x over previous
"""Trainium2 Bass kernel for geodesic convolution (gnn_message_passing).

Reference computation (per mesh vertex m, M=50000, n_in=n_out=32, grid 5x8):
  1. pullback[m,k,:] = sum_t bc_weights[m,k,t] * signal[bc_indices[m,k,t],:]
  2. x_grid[m,b,:]   = sum_{k: rad*8+ang==b} pullback[m,k,:]
  3. out_pre[m,r,o]  = sum_{i,j,n} x_grid[m,(i,j),n] * kernel[i,(j+r)%8,o,n]
  4. out[m,o]        = max_r relu(out_pre[m,r,o])

Data-parallel over m on 8 cores (6272 padded rows each, 49 tiles of 128
vertices), raw-bass pipeline (manual semaphores; the Ant dma_gather ucode is
incompatible with TileContext's event-semaphore machinery). Per tile:
  - ONE `dma_gather` (SWDGE ucode) of all 15360 (vertex, slot) indices
    fetches, per slot, the 128-byte fp16 row-PAIR holding the indexed signal
    row (pair id = idx>>1 fits the ucode's int16 index limit; rows are
    stored padded to 256B stride since the ucode's stride field is in 256B
    units; the wanted half is selected by folding idx&1 into per-half
    duplicated fp16 barycentric weights). One call per tile amortizes the
    ~1us fixed SWDGE descriptor-generation cost per call.
  - DVE: one tensor_tensor mult by the (slot, half) weights broadcast over
    32 channels, then ONE tensor_reduce folding (tap, half) -> f32 x_grid.
  - PE: per 128-wide contraction chunk, 1 transpose-matmul (m,(b,n)) ->
    ((b,n),m); then a 10-chunk K-accumulated fp16 matmul against the
    precomputed rotated-kernel matrix W2 (1280, 256) in (o, r) column
    order; ACT copies PSUM->SBUF (casting fp16 for the matmul lhsT).
  - DVE: tensor_reduce(max) over the 8 rotations straight out of PSUM
    (relu commutes with max; ACT applies it on the reduced [128,32] tile).

The grid scatter (step 2) is folded into the gather ordering: slots are
binned host-side by their (rad, ang) cell; round p gathers the p-th slot of
every bin (dummies gather pair 0 with weight 0). For the reference's meshgrid
rad/ang layout this is the identity ordering and R=1 (fast path). R>1 uses a
slower multi-round accumulation path.
"""

import os
from contextlib import ExitStack

import numpy as np

import concourse.bacc as bacc
import concourse.bass as bass
import concourse.mybir as mybir
from concourse.bass_utils import run_bass_kernel_spmd

M, N_IN, N_OUT = 50000, 32, 32
N_RHO, N_THETA = 5, 8
KV = N_RHO * N_THETA            # 40 grid bins
NS = KV * 3                     # 120 gather slots per vertex
N_CORES = 8
TILE_M = 128
TILES_PER_CORE = 49             # 49*128 = 6272 >= ceil(50000/8)
M_CORE = TILES_PER_CORE * TILE_M
M_PAD = N_CORES * M_CORE        # 50176
NCHUNK = (KV * N_IN) // 128     # 10 contraction chunks of 128
ROT_OUT = N_THETA * N_OUT       # 256
NIDX = TILE_M * NS              # 15360 gather indices per tile
IDXF = NIDX // 16               # 960 idx free-dim (16-partition wrap)
IDXP = 32                       # idx partitions read by the queue-0 ucode

f32 = mybir.dt.float32
f16 = mybir.dt.float16
i16 = mybir.dt.int16

last_exec_time_ns = None
last_result = None

_program_cache = {}


def _raw_gather(gp, out_ap, in_ap, idxs_ap, num_idxs, elem_size,
                stride_bytes_256):
    """dma_gather emitter without bass's elem_size%256 gate.

    The 256B element floor is a transpose-mode (xbar spray) restriction;
    the non-transpose ucode handles any elem_size, only the source row
    STRIDE must be a 256B multiple (stride_bytes_256 ISA field).
    """
    nb = gp.bass
    inst = gp.add_instruction(
        mybir.InstDMAGatherAnt(
            name=nb.get_next_instruction_name(),
            ins=[*gp.lower_ap_dma(in_ap, for_custom_bir_dma=True),
                 gp.lower_ap(idxs_ap),
                 gp.lower_val_access(gp.to_reg(num_idxs))],
            outs=[gp.lower_ap(out_ap)],
            transpose=False,
            num_idxs=num_idxs,
            elem_size=elem_size,
            stride_bytes_256=stride_bytes_256,
            gen_mode=0,
            single_packet=False,
            queue_num=0,
            sbuf_tokens_per_rank=0,
            sbuf_free_dim_per_rank=0,
            sbuf_free_dim_pad_per_rank=0,
            sbuf_byte_offset=0,
        ))
    return inst


def _build_program(n_rounds: int, n_tiles: int):
    nc = bacc.Bacc("TRN2", target_bir_lowering=False, debug=False,
                   num_devices=N_CORES, dynamic_dma_scratch_size=32768)

    R = n_rounds
    # signal: fp16 row-pairs padded to 256B stride (128B payload + 128B pad)
    sig_d = nc.dram_tensor("signal", [M // 2, 4 * N_IN], f16,
                           kind="ExternalInput")
    idx_d = nc.dram_tensor("idx", [R, n_tiles, IDXP, IDXF], i16,
                           kind="ExternalInput")
    wts_d = nc.dram_tensor("wts", [R, n_tiles, TILE_M, 2 * NS], f16,
                           kind="ExternalInput")
    w2_d = nc.dram_tensor("w2", [128, NCHUNK, ROT_OUT], f16,
                          kind="ExternalInput")
    ident_d = nc.dram_tensor("identity", [128, 128], f16, kind="ExternalInput")
    out_d = nc.dram_tensor("out", [n_tiles, TILE_M, N_OUT], f32,
                           kind="ExternalOutput")

    Ns = n_tiles * R  # gather steps

    with ExitStack() as ctx:
        e = ctx.enter_context

        def sb(name, shape, dt=f32):
            return e(nc.sbuf_tensor(name, shape, dt))

        gbuf = [sb(f"g{i}", [TILE_M, NS, 2 * N_IN], f16) for i in range(2)]
        idxb = [sb(f"idx{i}", [IDXP, IDXF], i16) for i in range(2)]
        wtsb = [sb(f"wts{i}", [TILE_M, 2 * NS], f16) for i in range(2)]
        phb = sb("ph", [TILE_M, NS, N_IN], f16)   # half-folded taps
        xgb = [sb(f"xg{i}", [TILE_M, KV, N_IN], f16) for i in range(2)]
        if R > 1:
            xtmp = sb("xtmp", [TILE_M, KV, N_IN], f16)
        xtb = [sb(f"xt{i}", [128, NCHUNK, 128], f16) for i in range(2)]
        rtb = [sb(f"rt{i}", [TILE_M, N_OUT]) for i in range(2)]
        otb = [sb(f"ot{i}", [TILE_M, N_OUT]) for i in range(2)]
        w2sb = sb("w2sb", [128, NCHUNK, ROT_OUT], f16)
        ident = sb("ident", [128, 128], f16)
        pstb = [e(nc.psum_tensor(f"pst{i}", [128, 128], f16)) for i in range(2)]
        opsb = [e(nc.psum_tensor(f"ops{i}", [TILE_M, ROT_OUT], f32))
                for i in range(2)]

        block = e(nc.Block())
        s_idx = [e(nc.semaphore(f"s_idx{i}")) for i in range(2)]
        s_wts = [e(nc.semaphore(f"s_wts{i}")) for i in range(2)]
        s_g = [e(nc.semaphore(f"s_g{i}")) for i in range(2)]
        s_out = [e(nc.semaphore(f"s_out{i}")) for i in range(2)]
        s_mult = e(nc.semaphore("s_mult"))
        s_red = e(nc.semaphore("s_red"))    # h-fold done (gbuf free)
        s_xg = e(nc.semaphore("s_xg"))      # t-fold done (xg ready)
        s_tp = e(nc.semaphore("s_tp"))      # per-chunk transpose done
        s_xt = e(nc.semaphore("s_xt"))      # per-chunk PSUM->SBUF copy done
        s_mm = e(nc.semaphore("s_mm"))      # main matmul done
        s_rm = e(nc.semaphore("s_rm"))      # rotation-max done (ops free)
        s_relu = e(nc.semaphore("s_relu"))  # relu done (out tile ready)
        s_w2 = e(nc.semaphore("s_w2"))
        s_id = e(nc.semaphore("s_id"))

        def w(eng, sem, val):
            if val > 0:
                eng.wait_ge(sem, val)

        # ---- SP sequencer: input + output DMA ----
        @block.sync
        def _(sp):
            sp.dma_start(out=w2sb[:], in_=w2_d[:]).then_inc(s_w2, 16)
            sp.dma_start(out=ident[:], in_=ident_d[:]).then_inc(s_id, 16)
            for q in range(Ns):
                t, r = divmod(q, R)
                # idx buf q%2: gather q-2 must have retired
                w(sp, s_g[q % 2], 16 * (q // 2))
                sp.dma_start(out=idxb[q % 2][:], in_=idx_d[r, t]
                             ).then_inc(s_idx[q % 2], 16)
                # wts buf q%2: mult q-2 must be done
                w(sp, s_mult, q - 1)
                sp.dma_start(out=wtsb[q % 2][:], in_=wts_d[r, t]
                             ).then_inc(s_wts[q % 2], 16)
                if r == R - 1 and t >= 2:
                    t_o = t - 2
                    w(sp, s_relu, t_o + 1)
                    sp.dma_start(out=out_d[t_o], in_=otb[t_o % 2][:]
                                 ).then_inc(s_out[t_o % 2], 16)
            for t_o in (n_tiles - 2, n_tiles - 1):
                w(sp, s_relu, t_o + 1)
                sp.dma_start(out=out_d[t_o], in_=otb[t_o % 2][:]
                             ).then_inc(s_out[t_o % 2], 16)
            sp.wait_ge(s_out[0], 16 * ((n_tiles + 1) // 2))
            sp.wait_ge(s_out[1], 16 * (n_tiles // 2))

        # ---- Pool: gathers ----
        @block.gpsimd
        def _(gp):
            for q in range(Ns):
                w(gp, s_idx[q % 2], 16 * (q // 2 + 1))
                # g buf q%2 free (fold q-2 done; s_red incs once per q)
                w(gp, s_red, q - 1)
                _raw_gather(gp, gbuf[q % 2][:, :, :], sig_d[:, 0:2 * N_IN],
                            idxb[q % 2][:, :], NIDX, 2 * N_IN,
                            stride_bytes_256=1
                            ).then_inc(s_g[q % 2], 16)

        # ---- DVE: weight mult, (tap, half) fold, rotation max ----
        @block.vector
        def _(dv):
            for q in range(Ns):
                t, r = divmod(q, R)
                g = gbuf[q % 2]
                w(dv, s_g[q % 2], 16 * (q // 2 + 1))
                w(dv, s_wts[q % 2], 16 * (q // 2 + 1))
                gv = g[:].rearrange("p s (h n) -> p (s h) n", n=N_IN)
                wb = wtsb[q % 2][:].to_broadcast([TILE_M, 2 * NS, N_IN])
                nc.vector.tensor_tensor(out=gv, in0=gv, in1=wb,
                                        op=mybir.AluOpType.mult
                                        ).then_inc(s_mult, 1)
                # fold halves into the contiguous ph buffer (in-place
                # strided dst measured 9x slower), freeing gbuf for the
                # next gather; then fold taps into xg
                g5 = g[:].rearrange("p (b t) (h n) -> p b t h n", t=3, n=N_IN)
                ph3 = phb[:].rearrange("p (b t) n -> p b t n", t=3)
                nc.vector.tensor_tensor(
                    out=phb[:], in0=g5[:, :, :, 0, :].rearrange(
                        "p b t n -> p (b t) n"),
                    in1=g5[:, :, :, 1, :].rearrange("p b t n -> p (b t) n"),
                    op=mybir.AluOpType.add).then_inc(s_red, 1)
                if r == 0:
                    w(dv, s_tp, NCHUNK * (t - 1))  # xg buf free
                dst = xgb[t % 2] if r == 0 else xtmp
                nc.vector.tensor_tensor(
                    out=dst[:], in0=ph3[:, :, 0, :], in1=ph3[:, :, 1, :],
                    op=mybir.AluOpType.add)
                ins = nc.vector.tensor_tensor(
                    out=dst[:], in0=dst[:], in1=ph3[:, :, 2, :],
                    op=mybir.AluOpType.add)
                if r > 0:
                    ins = nc.vector.tensor_tensor(
                        out=xgb[t % 2][:], in0=xgb[t % 2][:],
                        in1=xtmp[:], op=mybir.AluOpType.add)
                ins.then_inc(s_xg, 1)
                if r == R - 1 and t >= 1:
                    tm = t - 1
                    dv.wait_ge(s_mm, tm + 1)
                    w(dv, s_relu, tm - 1)  # rt buf free
                    nc.vector.tensor_reduce(
                        out=rtb[tm % 2][:],
                        in_=opsb[tm % 2][:].rearrange("p (o r) -> p o r",
                                                      r=N_THETA),
                        axis=mybir.AxisListType.X,
                        op=mybir.AluOpType.max).then_inc(s_rm, 1)
            tm = n_tiles - 1
            dv.wait_ge(s_mm, tm + 1)
            w(dv, s_relu, tm - 1)
            nc.vector.tensor_reduce(
                out=rtb[tm % 2][:],
                in_=opsb[tm % 2][:].rearrange("p (o r) -> p o r", r=N_THETA),
                axis=mybir.AxisListType.X,
                op=mybir.AluOpType.max).then_inc(s_rm, 1)

        # ---- PE: transpose + matmuls ----
        @block.tensor
        def _(pe):
            pe.wait_ge(s_id, 16)
            pe.wait_ge(s_w2, 16)
            for t in range(n_tiles):
                xg2 = xgb[t % 2][:].rearrange("p k n -> p (k n)")
                # fold of tile t fully done
                w(pe, s_xg, R * (t + 1))
                for c in range(NCHUNK):
                    G = NCHUNK * t + c
                    pst = pstb[G % 2]
                    w(pe, s_xt, G - 1)  # pst free: copy of chunk G-2 done
                    nc.tensor.matmul(
                        out=pst[:], lhsT=xg2[:, c * 128:(c + 1) * 128],
                        rhs=ident[:], is_transpose=True,
                        start=True, stop=True).then_inc(s_tp, 1)
                w(pe, s_xt, NCHUNK * (t + 1))
                w(pe, s_rm, t - 1)  # ops buf free
                ops = opsb[t % 2]
                for c in range(NCHUNK):
                    ins = nc.tensor.matmul(out=ops[:], lhsT=xtb[t % 2][:, c, :],
                                           rhs=w2sb[:, c, :],
                                           start=(c == 0),
                                           stop=(c == NCHUNK - 1))
                    if c == NCHUNK - 1:
                        ins.then_inc(s_mm, 1)

        # ---- ACT: PSUM->SBUF copies (f32 -> fp16 cast) + relu ----
        @block.scalar
        def _(ac):
            for t in range(n_tiles):
                for c in range(NCHUNK):
                    G = NCHUNK * t + c
                    w(ac, s_tp, G + 1)
                    w(ac, s_mm, t - 1)  # xt buf free
                    nc.scalar.copy(out=xtb[t % 2][:, c, :],
                                   in_=pstb[G % 2][:]).then_inc(s_xt, 1)
                if t >= 1:
                    tm = t - 1
                    w(ac, s_rm, tm + 1)
                    w(ac, s_out[tm % 2], 16 * (tm // 2))  # ot buf free
                    nc.scalar.activation(
                        out=otb[tm % 2][:], in_=rtb[tm % 2][:],
                        func=mybir.ActivationFunctionType.Relu
                        ).then_inc(s_relu, 1)
            tm = n_tiles - 1
            w(ac, s_rm, tm + 1)
            w(ac, s_out[tm % 2], 16 * (tm // 2))
            nc.scalar.activation(
                out=otb[tm % 2][:], in_=rtb[tm % 2][:],
                func=mybir.ActivationFunctionType.Relu).then_inc(s_relu, 1)

    nc.compile()
    return nc


def _build_w2(kernel):
    # W2[(i*8+j)*32+n, o*8+r] = kernel[i, (j+r)%8, o, n]
    # (o, r) column order so the rotation axis is innermost for the
    # DVE tensor_reduce(max) over r.
    k_rot = np.stack([np.roll(kernel, -r, axis=1) for r in range(N_THETA)], axis=0)
    w2 = k_rot.transpose(1, 2, 4, 3, 0).reshape(KV * N_IN, ROT_OUT)
    return np.ascontiguousarray(
        w2.reshape(NCHUNK, 128, ROT_OUT).transpose(1, 0, 2)).astype(np.float16)


def _build_rounds(bc_indices, bc_weights, rad_idx, ang_idx):
    flat = rad_idx.astype(np.int64) * N_THETA + ang_idx.astype(np.int64)
    if np.array_equal(flat, np.broadcast_to(np.arange(KV), flat.shape)):
        return (np.ascontiguousarray(bc_indices, dtype=np.int32)[None],
                np.ascontiguousarray(bc_weights, dtype=np.float32)[None])
    order = np.argsort(flat, axis=1, kind="stable")
    fs = np.take_along_axis(flat, order, axis=1)
    pos = np.broadcast_to(np.arange(KV), fs.shape)
    is_start = np.ones_like(fs, dtype=bool)
    is_start[:, 1:] = fs[:, 1:] != fs[:, :-1]
    start_pos = np.maximum.accumulate(np.where(is_start, pos, 0), axis=1)
    rank = (pos - start_pos).astype(np.int64)
    n_rounds = int(rank.max()) + 1
    bi_s = np.take_along_axis(bc_indices, order[:, :, None], axis=1)
    bw_s = np.take_along_axis(bc_weights, order[:, :, None], axis=1)
    m = flat.shape[0]
    gidx = np.zeros((n_rounds, m, KV, 3), dtype=np.int32)
    gw = np.zeros((n_rounds, m, KV, 3), dtype=np.float32)
    mm = np.broadcast_to(np.arange(m)[:, None], fs.shape)
    gidx[rank.ravel(), mm.ravel(), fs.ravel()] = bi_s.reshape(-1, 3)
    gw[rank.ravel(), mm.ravel(), fs.ravel()] = bw_s.reshape(-1, 3)
    return gidx, gw


def _prep_inputs(gidx, gw):
    """(R, M, KV, 3) idx/weights -> device idx16 (16-wrap, 2 replicas) +
    dual-half fp16 weights: idx16 (n_cores, R, n_tiles, IDXP, IDXF) i16,
    wts (n_cores, R, n_tiles, 128, 240) f16."""
    n_rounds = gidx.shape[0]
    gidx_p = np.zeros((n_rounds, M_PAD, NS), dtype=np.int32)
    gw_p = np.zeros((n_rounds, M_PAD, NS), dtype=np.float32)
    gidx_p[:, :M] = gidx.reshape(n_rounds, M, NS)
    gw_p[:, :M] = gw.reshape(n_rounds, M, NS)

    pair = (gidx_p >> 1).astype(np.int16)
    half = (gidx_p & 1).astype(np.float32)
    wts = np.empty((n_rounds, M_PAD, NS, 2), dtype=np.float32)
    wts[..., 0] = gw_p * (1.0 - half)
    wts[..., 1] = gw_p * half
    wts = wts.reshape(n_rounds, N_CORES, TILES_PER_CORE, TILE_M, 2 * NS)
    wts = np.ascontiguousarray(wts.transpose(1, 0, 2, 3, 4)).astype(np.float16)

    # gather order i = s*128 + m -> per-tile flat list (NS, 128)
    pair = pair.reshape(n_rounds, N_CORES, TILES_PER_CORE, TILE_M, NS)
    idx_flat = pair.transpose(1, 0, 2, 4, 3).reshape(
        N_CORES, n_rounds, TILES_PER_CORE, NIDX)
    # 16-partition wrap, replicated to the 2 groups of 16 partitions the
    # queue-0 ucode cores read
    wrap = idx_flat.reshape(N_CORES, n_rounds, TILES_PER_CORE, IDXF, 16)
    wrap = wrap.transpose(0, 1, 2, 4, 3)  # (.., 16, IDXF)
    idx16 = np.ascontiguousarray(
        np.broadcast_to(wrap[:, :, :, None, :, :],
                        (N_CORES, n_rounds, TILES_PER_CORE, IDXP // 16, 16,
                         IDXF))
        .reshape(N_CORES, n_rounds, TILES_PER_CORE, IDXP, IDXF))
    return idx16, wts


def kernel(signal, kernel, bc_weights, bc_indices, rad_idx, ang_idx):
    global last_exec_time_ns, last_result
    signal = np.asarray(signal, dtype=np.float32)
    # fp16 row-pairs padded to 256B stride: [25000, 128] f16, payload [:, :64]
    sig_pairs = np.zeros((M // 2, 4 * N_IN), dtype=np.float16)
    sig_pairs[:, :2 * N_IN] = signal.reshape(M // 2, 2 * N_IN).astype(np.float16)
    w2 = _build_w2(np.asarray(kernel, dtype=np.float32))
    gidx, gw = _build_rounds(np.asarray(bc_indices), np.asarray(bc_weights),
                             np.asarray(rad_idx), np.asarray(ang_idx))
    n_rounds = gidx.shape[0]
    idx16, wts = _prep_inputs(gidx, gw)

    key = (n_rounds, TILES_PER_CORE)
    if key not in _program_cache:
        _program_cache[key] = _build_program(n_rounds, TILES_PER_CORE)
    nc = _program_cache[key]

    ident = np.eye(128, dtype=np.float16)
    in_maps = [{"signal": sig_pairs, "idx": idx16[c], "wts": wts[c], "w2": w2,
                "identity": ident}
               for c in range(N_CORES)]

    trace = bool(int(os.environ.get("BASS_KERNEL_TRACE", "0")))
    kwargs = {}
    if trace:
        import prof_shim
        prof_shim.install()
        tdir = os.environ.get("BASS_KERNEL_TRACE_DIR")
        if tdir:
            os.makedirs(tdir, exist_ok=True)
            kwargs["tmpdir"] = tdir
    res = run_bass_kernel_spmd(nc, in_maps, core_ids=list(range(N_CORES)),
                               trace=trace, **kwargs)
    last_result = res
    last_exec_time_ns = res.exec_time_ns

    out = np.concatenate([res.results[c]["out"].reshape(M_CORE, N_OUT)
                          for c in range(N_CORES)], axis=0)
    return np.ascontiguousarray(out[:M])


# revision 36
# speedup vs baseline: 1.0321x; 1.0282x over previous
"""Trainium2 Bass kernel for geodesic convolution (gnn_message_passing).

Reference computation (per mesh vertex m, M=50000, n_in=n_out=32, grid 5x8):
  1. pullback[m,k,:] = sum_t bc_weights[m,k,t] * signal[bc_indices[m,k,t],:]
  2. x_grid[m,b,:]   = sum_{k: rad*8+ang==b} pullback[m,k,:]
  3. out_pre[m,r,o]  = sum_{i,j,n} x_grid[m,(i,j),n] * kernel[i,(j+r)%8,o,n]
  4. out[m,o]        = max_r relu(out_pre[m,r,o])

Data-parallel over m on 8 cores (6272 padded rows each, 49 tiles of 128
vertices), raw-bass pipeline (manual semaphores; the Ant dma_gather ucode is
incompatible with TileContext's event-semaphore machinery). Per tile:
  - ONE `dma_gather` (SWDGE ucode) of all 15360 (vertex, slot) indices
    fetches, per slot, the 128-byte fp16 row-PAIR holding the indexed signal
    row (pair id = idx>>1 fits the ucode's int16 index limit; rows are
    stored padded to 256B stride since the ucode's stride field is in 256B
    units; the wanted half is selected by folding idx&1 into per-half
    duplicated fp16 barycentric weights). One call per tile amortizes the
    ~1us fixed SWDGE descriptor-generation cost per call.
  - DVE: one tensor_tensor mult by the (slot, half) weights broadcast over
    32 channels, then ONE tensor_reduce folding (tap, half) -> f32 x_grid.
  - PE: per 128-wide contraction chunk, 1 transpose-matmul (m,(b,n)) ->
    ((b,n),m); then a 10-chunk K-accumulated fp16 matmul against the
    precomputed rotated-kernel matrix W2 (1280, 256) in (o, r) column
    order; ACT copies PSUM->SBUF (casting fp16 for the matmul lhsT).
  - DVE: tensor_reduce(max) over the 8 rotations straight out of PSUM
    (relu commutes with max; ACT applies it on the reduced [128,32] tile).

The grid scatter (step 2) is folded into the gather ordering: slots are
binned host-side by their (rad, ang) cell; round p gathers the p-th slot of
every bin (dummies gather pair 0 with weight 0). For the reference's meshgrid
rad/ang layout this is the identity ordering and R=1 (fast path). R>1 uses a
slower multi-round accumulation path.
"""

import os
from contextlib import ExitStack

import numpy as np

import concourse.bacc as bacc
import concourse.bass as bass
import concourse.mybir as mybir
from concourse.bass_utils import run_bass_kernel_spmd

M, N_IN, N_OUT = 50000, 32, 32
N_RHO, N_THETA = 5, 8
KV = N_RHO * N_THETA            # 40 grid bins
NS = KV * 3                     # 120 gather slots per vertex
N_CORES = 8
TILE_M = 128
TILES_PER_CORE = 49             # 49*128 = 6272 >= ceil(50000/8)
M_CORE = TILES_PER_CORE * TILE_M
M_PAD = N_CORES * M_CORE        # 50176
NCHUNK = (KV * N_IN) // 128     # 10 contraction chunks of 128
ROT_OUT = N_THETA * N_OUT       # 256
NIDX = TILE_M * NS              # 15360 gather indices per tile
IDXF = NIDX // 16               # 960 idx free-dim (16-partition wrap)
IDXP = 32                       # idx partitions read by the queue-0 ucode

f32 = mybir.dt.float32
f16 = mybir.dt.float16
i16 = mybir.dt.int16

last_exec_time_ns = None
last_result = None

_program_cache = {}


def _raw_gather(gp, out_ap, in_ap, idxs_ap, num_idxs, elem_size,
                stride_bytes_256):
    """dma_gather emitter without bass's elem_size%256 gate.

    The 256B element floor is a transpose-mode (xbar spray) restriction;
    the non-transpose ucode handles any elem_size, only the source row
    STRIDE must be a 256B multiple (stride_bytes_256 ISA field).
    """
    nb = gp.bass
    inst = gp.add_instruction(
        mybir.InstDMAGatherAnt(
            name=nb.get_next_instruction_name(),
            ins=[*gp.lower_ap_dma(in_ap, for_custom_bir_dma=True),
                 gp.lower_ap(idxs_ap),
                 gp.lower_val_access(gp.to_reg(num_idxs))],
            outs=[gp.lower_ap(out_ap)],
            transpose=False,
            num_idxs=num_idxs,
            elem_size=elem_size,
            stride_bytes_256=stride_bytes_256,
            gen_mode=0,
            single_packet=False,
            queue_num=0,
            sbuf_tokens_per_rank=0,
            sbuf_free_dim_per_rank=0,
            sbuf_free_dim_pad_per_rank=0,
            sbuf_byte_offset=0,
        ))
    return inst


def _build_program(n_rounds: int, n_tiles: int):
    nc = bacc.Bacc("TRN2", target_bir_lowering=False, debug=False,
                   num_devices=N_CORES, dynamic_dma_scratch_size=32768)

    R = n_rounds
    # signal: fp16 row-pairs padded to 256B stride (128B payload + 128B pad)
    sig_d = nc.dram_tensor("signal", [M // 2, 4 * N_IN], f16,
                           kind="ExternalInput")
    idx_d = nc.dram_tensor("idx", [R, n_tiles, IDXP, IDXF], i16,
                           kind="ExternalInput")
    wts_d = nc.dram_tensor("wts", [R, n_tiles, TILE_M, 2 * NS], f16,
                           kind="ExternalInput")
    w2_d = nc.dram_tensor("w2", [128, NCHUNK, ROT_OUT], f16,
                          kind="ExternalInput")
    ident_d = nc.dram_tensor("identity", [128, 128], f16, kind="ExternalInput")
    out_d = nc.dram_tensor("out", [n_tiles, TILE_M, N_OUT], f32,
                           kind="ExternalOutput")

    Ns = n_tiles * R  # gather steps

    with ExitStack() as ctx:
        e = ctx.enter_context

        def sb(name, shape, dt=f32):
            return e(nc.sbuf_tensor(name, shape, dt))

        gbuf = [sb(f"g{i}", [TILE_M, NS, 2 * N_IN], f16) for i in range(2)]
        idxb = [sb(f"idx{i}", [IDXP, IDXF], i16) for i in range(2)]
        wtsb = [sb(f"wts{i}", [TILE_M, 2 * NS], f16) for i in range(2)]
        phb = sb("ph", [TILE_M, KV, 2 * N_IN], f16)  # tap-folded (n,h) pairs
        xgb = [sb(f"xg{i}", [TILE_M, KV, N_IN], f16) for i in range(4)]
        if R > 1:
            xtmp = sb("xtmp", [TILE_M, KV, N_IN], f16)
        xtb = [sb(f"xt{i}", [128, NCHUNK, 128], f16) for i in range(2)]
        rtb = [sb(f"rt{i}", [TILE_M, N_OUT]) for i in range(2)]
        otb = [sb(f"ot{i}", [TILE_M, N_OUT]) for i in range(2)]
        w2sb = sb("w2sb", [128, NCHUNK, ROT_OUT], f16)
        ident = sb("ident", [128, 128], f16)
        pstb = [e(nc.psum_tensor(f"pst{i}", [128, 128], f16)) for i in range(4)]
        opsb = [e(nc.psum_tensor(f"ops{i}", [TILE_M, ROT_OUT], f32))
                for i in range(2)]

        block = e(nc.Block())
        s_idx = [e(nc.semaphore(f"s_idx{i}")) for i in range(2)]
        s_wts = [e(nc.semaphore(f"s_wts{i}")) for i in range(2)]
        s_g = [e(nc.semaphore(f"s_g{i}")) for i in range(2)]
        s_out = [e(nc.semaphore(f"s_out{i}")) for i in range(2)]
        s_mult = e(nc.semaphore("s_mult"))
        s_red = e(nc.semaphore("s_red"))    # h-fold done (gbuf free)
        s_xg = e(nc.semaphore("s_xg"))      # t-fold done (xg ready)
        s_tp = e(nc.semaphore("s_tp"))      # per-chunk transpose done
        s_xt = e(nc.semaphore("s_xt"))      # per-chunk PSUM->SBUF copy done
        s_mm = e(nc.semaphore("s_mm"))      # main matmul done
        s_rm = e(nc.semaphore("s_rm"))      # rotation-max done (ops free)
        s_relu = e(nc.semaphore("s_relu"))  # relu done (out tile ready)
        s_w2 = e(nc.semaphore("s_w2"))
        s_id = e(nc.semaphore("s_id"))

        def w(eng, sem, val):
            if val > 0:
                eng.wait_ge(sem, val)

        # ---- SP sequencer: input + output DMA ----
        @block.sync
        def _(sp):
            sp.dma_start(out=w2sb[:], in_=w2_d[:]).then_inc(s_w2, 16)
            sp.dma_start(out=ident[:], in_=ident_d[:]).then_inc(s_id, 16)
            for q in range(Ns):
                t, r = divmod(q, R)
                # idx buf q%2: gather q-2 must have retired
                w(sp, s_g[q % 2], 16 * (q // 2))
                sp.dma_start(out=idxb[q % 2][:], in_=idx_d[r, t]
                             ).then_inc(s_idx[q % 2], 16)
                # wts buf q%2: mult q-2 must be done
                w(sp, s_mult, q - 1)
                sp.dma_start(out=wtsb[q % 2][:], in_=wts_d[r, t]
                             ).then_inc(s_wts[q % 2], 16)
                if r == R - 1 and t >= 2:
                    t_o = t - 2
                    w(sp, s_relu, t_o + 1)
                    sp.dma_start(out=out_d[t_o], in_=otb[t_o % 2][:]
                                 ).then_inc(s_out[t_o % 2], 16)
            for t_o in (n_tiles - 2, n_tiles - 1):
                w(sp, s_relu, t_o + 1)
                sp.dma_start(out=out_d[t_o], in_=otb[t_o % 2][:]
                             ).then_inc(s_out[t_o % 2], 16)
            sp.wait_ge(s_out[0], 16 * ((n_tiles + 1) // 2))
            sp.wait_ge(s_out[1], 16 * (n_tiles // 2))

        # ---- Pool: gathers ----
        @block.gpsimd
        def _(gp):
            for q in range(Ns):
                w(gp, s_idx[q % 2], 16 * (q // 2 + 1))
                # g buf q%2 free (fold q-2 done; s_red incs once per q)
                w(gp, s_red, q - 1)
                _raw_gather(gp, gbuf[q % 2][:, :, :], sig_d[:, 0:2 * N_IN],
                            idxb[q % 2][:, :], NIDX, 2 * N_IN,
                            stride_bytes_256=1
                            ).then_inc(s_g[q % 2], 16)

        # ---- DVE: weight mult, (tap, half) fold, rotation max ----
        @block.vector
        def _(dv):
            for q in range(Ns):
                t, r = divmod(q, R)
                g = gbuf[q % 2]
                w(dv, s_g[q % 2], 16 * (q // 2 + 1))
                w(dv, s_wts[q % 2], 16 * (q // 2 + 1))
                # signal pairs are stored (n, h)-interleaved: the weight
                # broadcast then has stride-0 only on a middle dim, so every
                # AP's innermost step is 1 and the DVE's 2-per-cycle fp16
                # mode engages for both the mult and the half-fold reduce.
                gv4 = g[:].rearrange("p s (n h) -> p s n h", h=2)
                wb4 = (wtsb[q % 2][:].rearrange("p (s h) -> p s h", h=2)
                       .to_broadcast([TILE_M, NS, 2, N_IN])
                       .rearrange("p s h n -> p s n h"))
                nc.vector.tensor_tensor(out=gv4, in0=gv4, in1=wb4,
                                        op=mybir.AluOpType.mult
                                        ).then_inc(s_mult, 1)
                # fold taps first (64-elem runs at 384B stride stay off the
                # pathological 128B SBUF bank period), freeing gbuf; then
                # reduce the interleaved halves contiguously into xg
                gt = g[:].rearrange("p (b t) e -> p b t e", t=3)
                nc.vector.tensor_tensor(
                    out=phb[:], in0=gt[:, :, 0, :], in1=gt[:, :, 1, :],
                    op=mybir.AluOpType.add)
                nc.vector.tensor_tensor(
                    out=phb[:], in0=phb[:], in1=gt[:, :, 2, :],
                    op=mybir.AluOpType.add).then_inc(s_red, 1)
                if r == 0:
                    w(dv, s_tp, NCHUNK * (t - 3))  # xg buf free (4 bufs)
                dst = xgb[t % 4] if r == 0 else xtmp
                ins = None
                with nc.allow_low_precision(reason="2-term fp16 half fold"):
                    ins = nc.vector.tensor_reduce(
                        out=dst[:],
                        in_=phb[:].rearrange("p b (n h) -> p b n h", h=2),
                        axis=mybir.AxisListType.X, op=mybir.AluOpType.add)
                if r > 0:
                    ins = nc.vector.tensor_tensor(
                        out=xgb[t % 4][:], in0=xgb[t % 4][:],
                        in1=xtmp[:], op=mybir.AluOpType.add)
                ins.then_inc(s_xg, 1)
                if r == R - 1 and t >= 1:
                    tm = t - 1
                    dv.wait_ge(s_mm, tm + 1)
                    w(dv, s_relu, tm - 1)  # rt buf free
                    nc.vector.tensor_reduce(
                        out=rtb[tm % 2][:],
                        in_=opsb[tm % 2][:].rearrange("p (o r) -> p o r",
                                                      r=N_THETA),
                        axis=mybir.AxisListType.X,
                        op=mybir.AluOpType.max).then_inc(s_rm, 1)
            tm = n_tiles - 1
            dv.wait_ge(s_mm, tm + 1)
            w(dv, s_relu, tm - 1)
            nc.vector.tensor_reduce(
                out=rtb[tm % 2][:],
                in_=opsb[tm % 2][:].rearrange("p (o r) -> p o r", r=N_THETA),
                axis=mybir.AxisListType.X,
                op=mybir.AluOpType.max).then_inc(s_rm, 1)

        # ---- PE: transpose + matmuls ----
        @block.tensor
        def _(pe):
            pe.wait_ge(s_id, 16)
            pe.wait_ge(s_w2, 16)
            for t in range(n_tiles):
                xg2 = xgb[t % 4][:].rearrange("p k n -> p (k n)")
                # fold of tile t fully done
                w(pe, s_xg, R * (t + 1))
                for c in range(NCHUNK):
                    G = NCHUNK * t + c
                    pst = pstb[G % 4]
                    w(pe, s_xt, G - 3)  # pst free: copy of chunk G-4 done
                    nc.tensor.matmul(
                        out=pst[:], lhsT=xg2[:, c * 128:(c + 1) * 128],
                        rhs=ident[:], is_transpose=True,
                        start=True, stop=True).then_inc(s_tp, 1)
                w(pe, s_xt, NCHUNK * (t + 1))
                w(pe, s_rm, t - 1)  # ops buf free
                ops = opsb[t % 2]
                for c in range(NCHUNK):
                    ins = nc.tensor.matmul(out=ops[:], lhsT=xtb[t % 2][:, c, :],
                                           rhs=w2sb[:, c, :],
                                           start=(c == 0),
                                           stop=(c == NCHUNK - 1))
                    if c == NCHUNK - 1:
                        ins.then_inc(s_mm, 1)

        # ---- ACT: PSUM->SBUF copies (f32 -> fp16 cast) + relu ----
        @block.scalar
        def _(ac):
            for t in range(n_tiles):
                for c in range(NCHUNK):
                    G = NCHUNK * t + c
                    w(ac, s_tp, G + 1)
                    w(ac, s_mm, t - 1)  # xt buf free
                    nc.scalar.copy(out=xtb[t % 2][:, c, :],
                                   in_=pstb[G % 4][:]).then_inc(s_xt, 1)
                if t >= 1:
                    tm = t - 1
                    w(ac, s_rm, tm + 1)
                    w(ac, s_out[tm % 2], 16 * (tm // 2))  # ot buf free
                    nc.scalar.activation(
                        out=otb[tm % 2][:], in_=rtb[tm % 2][:],
                        func=mybir.ActivationFunctionType.Relu
                        ).then_inc(s_relu, 1)
            tm = n_tiles - 1
            w(ac, s_rm, tm + 1)
            w(ac, s_out[tm % 2], 16 * (tm // 2))
            nc.scalar.activation(
                out=otb[tm % 2][:], in_=rtb[tm % 2][:],
                func=mybir.ActivationFunctionType.Relu).then_inc(s_relu, 1)

    nc.compile()
    return nc


def _build_w2(kernel):
    # W2[(i*8+j)*32+n, o*8+r] = kernel[i, (j+r)%8, o, n]
    # (o, r) column order so the rotation axis is innermost for the
    # DVE tensor_reduce(max) over r.
    k_rot = np.stack([np.roll(kernel, -r, axis=1) for r in range(N_THETA)], axis=0)
    w2 = k_rot.transpose(1, 2, 4, 3, 0).reshape(KV * N_IN, ROT_OUT)
    return np.ascontiguousarray(
        w2.reshape(NCHUNK, 128, ROT_OUT).transpose(1, 0, 2)).astype(np.float16)


def _build_rounds(bc_indices, bc_weights, rad_idx, ang_idx):
    flat = rad_idx.astype(np.int64) * N_THETA + ang_idx.astype(np.int64)
    if np.array_equal(flat, np.broadcast_to(np.arange(KV), flat.shape)):
        return (np.ascontiguousarray(bc_indices, dtype=np.int32)[None],
                np.ascontiguousarray(bc_weights, dtype=np.float32)[None])
    order = np.argsort(flat, axis=1, kind="stable")
    fs = np.take_along_axis(flat, order, axis=1)
    pos = np.broadcast_to(np.arange(KV), fs.shape)
    is_start = np.ones_like(fs, dtype=bool)
    is_start[:, 1:] = fs[:, 1:] != fs[:, :-1]
    start_pos = np.maximum.accumulate(np.where(is_start, pos, 0), axis=1)
    rank = (pos - start_pos).astype(np.int64)
    n_rounds = int(rank.max()) + 1
    bi_s = np.take_along_axis(bc_indices, order[:, :, None], axis=1)
    bw_s = np.take_along_axis(bc_weights, order[:, :, None], axis=1)
    m = flat.shape[0]
    gidx = np.zeros((n_rounds, m, KV, 3), dtype=np.int32)
    gw = np.zeros((n_rounds, m, KV, 3), dtype=np.float32)
    mm = np.broadcast_to(np.arange(m)[:, None], fs.shape)
    gidx[rank.ravel(), mm.ravel(), fs.ravel()] = bi_s.reshape(-1, 3)
    gw[rank.ravel(), mm.ravel(), fs.ravel()] = bw_s.reshape(-1, 3)
    return gidx, gw


def _prep_inputs(gidx, gw):
    """(R, M, KV, 3) idx/weights -> device idx16 (16-wrap, 2 replicas) +
    dual-half fp16 weights: idx16 (n_cores, R, n_tiles, IDXP, IDXF) i16,
    wts (n_cores, R, n_tiles, 128, 240) f16."""
    n_rounds = gidx.shape[0]
    gidx_p = np.zeros((n_rounds, M_PAD, NS), dtype=np.int32)
    gw_p = np.zeros((n_rounds, M_PAD, NS), dtype=np.float32)
    gidx_p[:, :M] = gidx.reshape(n_rounds, M, NS)
    gw_p[:, :M] = gw.reshape(n_rounds, M, NS)

    pair = (gidx_p >> 1).astype(np.int16)
    half = (gidx_p & 1).astype(np.float32)
    wts = np.empty((n_rounds, M_PAD, NS, 2), dtype=np.float32)
    wts[..., 0] = gw_p * (1.0 - half)
    wts[..., 1] = gw_p * half
    wts = wts.reshape(n_rounds, N_CORES, TILES_PER_CORE, TILE_M, 2 * NS)
    wts = np.ascontiguousarray(wts.transpose(1, 0, 2, 3, 4)).astype(np.float16)

    # gather order i = s*128 + m -> per-tile flat list (NS, 128)
    pair = pair.reshape(n_rounds, N_CORES, TILES_PER_CORE, TILE_M, NS)
    idx_flat = pair.transpose(1, 0, 2, 4, 3).reshape(
        N_CORES, n_rounds, TILES_PER_CORE, NIDX)
    # 16-partition wrap, replicated to the 2 groups of 16 partitions the
    # queue-0 ucode cores read
    wrap = idx_flat.reshape(N_CORES, n_rounds, TILES_PER_CORE, IDXF, 16)
    wrap = wrap.transpose(0, 1, 2, 4, 3)  # (.., 16, IDXF)
    idx16 = np.ascontiguousarray(
        np.broadcast_to(wrap[:, :, :, None, :, :],
                        (N_CORES, n_rounds, TILES_PER_CORE, IDXP // 16, 16,
                         IDXF))
        .reshape(N_CORES, n_rounds, TILES_PER_CORE, IDXP, IDXF))
    return idx16, wts


def kernel(signal, kernel, bc_weights, bc_indices, rad_idx, ang_idx):
    global last_exec_time_ns, last_result
    signal = np.asarray(signal, dtype=np.float32)
    # fp16 row-pairs, (n, h)-interleaved within each pair, padded to 256B
    # stride: [25000, 128] f16, payload [:, :64]
    inter = signal.reshape(M // 2, 2, N_IN).transpose(0, 2, 1)
    sig_pairs = np.zeros((M // 2, 4 * N_IN), dtype=np.float16)
    sig_pairs[:, :2 * N_IN] = inter.reshape(M // 2, 2 * N_IN).astype(np.float16)
    w2 = _build_w2(np.asarray(kernel, dtype=np.float32))
    gidx, gw = _build_rounds(np.asarray(bc_indices), np.asarray(bc_weights),
                             np.asarray(rad_idx), np.asarray(ang_idx))
    n_rounds = gidx.shape[0]
    idx16, wts = _prep_inputs(gidx, gw)

    key = (n_rounds, TILES_PER_CORE)
    if key not in _program_cache:
        _program_cache[key] = _build_program(n_rounds, TILES_PER_CORE)
    nc = _program_cache[key]

    ident = np.eye(128, dtype=np.float16)
    in_maps = [{"signal": sig_pairs, "idx": idx16[c], "wts": wts[c], "w2": w2,
                "identity": ident}
               for c in range(N_CORES)]

    trace = bool(int(os.environ.get("BASS_KERNEL_TRACE", "0")))
    kwargs = {}
    if trace:
        import prof_shim
        prof_shim.install()
        tdir = os.environ.get("BASS_KERNEL_TRACE_DIR")
        if tdir:
            os.makedirs(tdir, exist_ok=True)
            kwargs["tmpdir"] = tdir
    res = run_bass_kernel_spmd(nc, in_maps, core_ids=list(range(N_CORES)),
                               trace=trace, **kwargs)
    last_result = res
    last_exec_time_ns = res.exec_time_ns

    out = np.concatenate([res.results[c]["out"].reshape(M_CORE, N_OUT)
                          for c in range(N_CORES)], axis=0)
    return np.ascontiguousarray(out[:M])
